# revision 1
# baseline (speedup 1.0000x reference)
"""MoE (63 routed experts top-7 + 1 shared expert) Trainium2 kernel.

Strategy: expert-parallel sparse dispatch. The router (softmax + top-k,
~0.3% of FLOPs) runs on host; tokens are gathered expert-major into
fixed-capacity weight slots, which are distributed across 8 NeuronCores.
Each core runs an identical (SPMD) Bass program: for every slot, a
1280->1280 Linear + exact GELU + 1280->1280 Linear over 1024 tokens,
feature-major (features on partitions, tokens on the free dim) so weights
need no transpose and biases ride the activation unit's per-partition
bias port. Outputs are gathered and gate-weighted back on host in the
reference's exact accumulation order.
"""

import os
import sys
import math

sys.path.insert(0, "/opt/trn_rl_repo")

import numpy as np

D = 1280          # model dim
I = 1280          # expert inter dim
EXPERTS = 63      # routed experts
TOPK = 7          # routed top-k
CAP = 1024        # tokens per weight slot
CHUNK = 512       # tokens per matmul (fp32 moving-operand max)
KT = D // 128     # 10 contraction tiles
NCORES = 8

MM_DTYPE = os.environ.get("MM_DTYPE", "fp16")   # "fp16" | "f32r" | "bf16"

_PROGRAM_CACHE = {}


# ----------------------------------------------------------------- router

def _route(x2d, wr, br):
    """f32 softmax + top-k, matching jax.nn.softmax / jax.lax.top_k."""
    logits = (x2d @ wr + br).astype(np.float32)
    logits -= logits.max(-1, keepdims=True)
    np.exp(logits, out=logits)
    aff = logits / logits.sum(-1, keepdims=True)
    idx = np.argsort(-aff, axis=-1, kind="stable")[:, :TOPK]
    vals = np.take_along_axis(aff, idx, axis=-1)
    return idx.astype(np.int32), vals.astype(np.float32)


def _build_plan(T, idx):
    """Pack (token, expert) pairs expert-major into CAP-token pieces, plus
    the shared expert's T tokens, into 8 cores x S slots."""
    flat = idx.ravel()
    order = np.argsort(flat, kind="stable")          # expert-major slot order
    tok_of = (order // TOPK).astype(np.int64)
    counts = np.bincount(flat, minlength=EXPERTS)
    offs = np.concatenate([[0], np.cumsum(counts)])

    pieces = []  # (kind, expert, a, b)  [a:b) into the expert-major order
    for e in range(EXPERTS):
        a, b = int(offs[e]), int(offs[e + 1])
        while a < b:
            n = min(CAP, b - a)
            pieces.append(("r", e, a, a + n))
            a += n

    n_shared_min = math.ceil(T / CAP)
    S = max(1, math.ceil((len(pieces) + n_shared_min) / NCORES))
    n_shared = NCORES * S - len(pieces)
    # split T shared tokens near-evenly over n_shared pieces (each <= CAP)
    base, rem = divmod(T, n_shared)
    assert base + (1 if rem else 0) <= CAP
    t0 = 0
    for j in range(n_shared):
        n = base + (1 if j < rem else 0)
        pieces.append(("s", -1, t0, t0 + n))
        t0 += n
    assert t0 == T and len(pieces) == NCORES * S
    return pieces, S, order, tok_of


# ----------------------------------------------------------- device program

def _build_program(S, M):
    import concourse.bass as bass
    import concourse.mybir as mybir
    import concourse.tile as tile
    from concourse import bacc

    f32 = mybir.dt.float32
    in_dt = {"bf16": mybir.dt.bfloat16,
             "fp16": mybir.dt.float16,
             "f32r": mybir.dt.float32r}[MM_DTYPE]

    nc = bacc.Bacc("TRN2", target_bir_lowering=False, debug=False,
                   enable_asserts=False, num_devices=NCORES)
    xT = nc.dram_tensor("xT", [KT, 128, M], in_dt, kind="ExternalInput").ap()
    w1s = nc.dram_tensor("w1s", [S, KT, 128, KT, 128], in_dt, kind="ExternalInput").ap()
    w2s = nc.dram_tensor("w2s", [S, KT, 128, KT, 128], in_dt, kind="ExternalInput").ap()
    b1s = nc.dram_tensor("b1s", [S, 128, KT], f32, kind="ExternalInput").ap()
    b2s = nc.dram_tensor("b2s", [S, 128, KT], f32, kind="ExternalInput").ap()
    yT = nc.dram_tensor("yT", [KT, 128, M], f32, kind="ExternalOutput").ap()

    CPS = CAP // CHUNK  # chunks per slot
    Gelu = mybir.ActivationFunctionType.Gelu
    Ident = mybir.ActivationFunctionType.Identity

    def mm_ap(ap):
        return ap

    with tile.TileContext(nc) as tc:
        with (
            tc.tile_pool(name="xa", bufs=3) as xa,
            tc.tile_pool(name="w1p", bufs=4) as w1p,
            tc.tile_pool(name="w2p", bufs=4) as w2p,
            tc.tile_pool(name="hp", bufs=3) as hp,
            tc.tile_pool(name="yo", bufs=6) as yo,
            tc.tile_pool(name="bp", bufs=2) as bp,
            tc.tile_pool(name="ps", bufs=8, space="PSUM") as ps,
        ):
            for s in range(S):
                col0 = s * CAP
                b1t = bp.tile([128, KT], f32, tag="b1", name="b1t")
                nc.sync.dma_start(out=b1t[:, :], in_=b1s[s])
                b2t = bp.tile([128, KT], f32, tag="b2", name="b2t")
                nc.sync.dma_start(out=b2t[:, :], in_=b2s[s])

                xc = []
                for c in range(CPS):
                    xt = xa.tile([128, KT, CHUNK], in_dt, tag="x", name="xt")
                    for k in range(KT):
                        # SWDGE: keeps HWDGE free for the slot's weight loads
                        nc.gpsimd.dma_start(
                            out=xt[:, k, :],
                            in_=xT[k, :, col0 + c * CHUNK: col0 + (c + 1) * CHUNK])
                    xc.append(xt)

                hc = [hp.tile([128, KT, CHUNK], in_dt, tag="h", name=f"h{c}")
                      for c in range(CPS)]

                # layer 1: h = gelu(x @ w1 + b1), feature-major
                for io in range(KT):
                    w1t = w1p.tile([128, KT, 128], in_dt, tag="w1", name="w1t")
                    nc.sync.dma_start(out=w1t[:, :, :], in_=w1s[s, io])
                    for c in range(CPS):
                        pt = ps.tile([128, CHUNK], f32, tag="ps", name="pt")
                        for k in range(KT):
                            nc.tensor.matmul(pt[:, :], mm_ap(w1t[:, k, :]),
                                             mm_ap(xc[c][:, k, :]),
                                             start=(k == 0), stop=(k == KT - 1))
                        nc.scalar.activation(hc[c][:, io, :], pt[:, :], Gelu,
                                             bias=b1t[:, io:io + 1])

                # layer 2: y = h @ w2 + b2
                for io in range(KT):
                    w2t = w2p.tile([128, KT, 128], in_dt, tag="w2", name="w2t")
                    nc.sync.dma_start(out=w2t[:, :, :], in_=w2s[s, io])
                    for c in range(CPS):
                        pt = ps.tile([128, CHUNK], f32, tag="ps", name="pt")
                        for k in range(KT):
                            nc.tensor.matmul(pt[:, :], mm_ap(w2t[:, k, :]),
                                             mm_ap(hc[c][:, k, :]),
                                             start=(k == 0), stop=(k == KT - 1))
                        yt = yo.tile([128, CHUNK], f32, tag="y", name="yt")
                        nc.scalar.activation(yt[:, :], pt[:, :], Ident,
                                             bias=b2t[:, io:io + 1])
                        nc.sync.dma_start(
                            out=yT[io, :, col0 + c * CHUNK: col0 + (c + 1) * CHUNK],
                            in_=yt[:, :])
    nc.compile()
    return nc


def _get_program(S, M):
    key = (S, M, MM_DTYPE)
    if key not in _PROGRAM_CACHE:
        _PROGRAM_CACHE[key] = _build_program(S, M)
    return _PROGRAM_CACHE[key]


# ------------------------------------------------------------------ kernel

def _np_dt():
    import ml_dtypes
    return {"bf16": ml_dtypes.bfloat16, "fp16": np.float16,
            "f32r": np.float32}[MM_DTYPE]


def _arrange_w(w):
    """[D, I] -> [io, p, ko, c] so each (slot, io) block DMAs contiguously
    into an SBUF tile laid out [partition, ko, col]."""
    return np.ascontiguousarray(
        w.reshape(KT, 128, KT, 128).transpose(2, 1, 0, 3))


def kernel(x, sw1, sb1, sw2, sb2, rw1, rb1, rw2, rb2, wr, br, _trace=False):
    from concourse.bass_utils import run_bass_kernel_spmd

    x = np.asarray(x, dtype=np.float32)
    B, Sq, _ = x.shape
    T = B * Sq
    xf = np.ascontiguousarray(x.reshape(T, D))

    idx, vals = _route(xf, np.asarray(wr, np.float32), np.asarray(br, np.float32))
    pieces, S, order, tok_of = _build_plan(T, idx)
    M = S * CAP
    dt = _np_dt()

    rw1 = np.asarray(rw1, np.float32); rw2 = np.asarray(rw2, np.float32)
    rb1 = np.asarray(rb1, np.float32); rb2 = np.asarray(rb2, np.float32)
    sw1 = np.asarray(sw1, np.float32); sw2 = np.asarray(sw2, np.float32)
    sb1 = np.asarray(sb1, np.float32); sb2 = np.asarray(sb2, np.float32)

    # pre-arranged weights, cached per id of the weight arrays
    w1a = [_arrange_w(rw1[e]).astype(dt) for e in range(EXPERTS)]
    w2a = [_arrange_w(rw2[e]).astype(dt) for e in range(EXPERTS)]
    sw1a = _arrange_w(sw1).astype(dt)
    sw2a = _arrange_w(sw2).astype(dt)
    b1a = [np.ascontiguousarray(rb1[e].reshape(KT, 128).T) for e in range(EXPERTS)]
    b2a = [np.ascontiguousarray(rb2[e].reshape(KT, 128).T) for e in range(EXPERTS)]
    sb1a = np.ascontiguousarray(sb1.reshape(KT, 128).T)
    sb2a = np.ascontiguousarray(sb2.reshape(KT, 128).T)

    xfT = np.ascontiguousarray(xf.T)  # [D, T]
    tok_r = tok_of  # token of each expert-major (token,k) pair

    in_maps = []
    for core in range(NCORES):
        xT_core = np.zeros((D, M), dtype=dt)
        w1_core = np.zeros((S, KT, 128, KT, 128), dtype=dt)
        w2_core = np.zeros((S, KT, 128, KT, 128), dtype=dt)
        b1_core = np.zeros((S, 128, KT), dtype=np.float32)
        b2_core = np.zeros((S, 128, KT), dtype=np.float32)
        for j in range(S):
            kind, e, a, b = pieces[core * S + j]
            toks = tok_r[a:b] if kind == "r" else np.arange(a, b)
            xT_core[:, j * CAP: j * CAP + (b - a)] = xfT[:, toks]
            if kind == "r":
                w1_core[j] = w1a[e]; w2_core[j] = w2a[e]
                b1_core[j] = b1a[e]; b2_core[j] = b2a[e]
            else:
                w1_core[j] = sw1a; w2_core[j] = sw2a
                b1_core[j] = sb1a; b2_core[j] = sb2a
        in_maps.append({
            "xT": xT_core.reshape(KT, 128, M),
            "w1s": w1_core, "w2s": w2_core,
            "b1s": b1_core, "b2s": b2_core,
        })

    nc = _get_program(S, M)
    res = run_bass_kernel_spmd(nc, in_maps, core_ids=list(range(NCORES)),
                               trace=_trace)
    kernel.last_result = res

    TK = T * TOPK
    gated = np.empty((TK, D), dtype=np.float32)   # expert-major rows
    shared_out = np.empty((T, D), dtype=np.float32)
    for core in range(NCORES):
        Y = res.results[core]["yT"].reshape(D, M)
        for j in range(S):
            kind, e, a, b = pieces[core * S + j]
            block = Y[:, j * CAP: j * CAP + (b - a)].T  # [n, D]
            if kind == "r":
                gated[a:b] = block
            else:
                shared_out[a:b] = block

    g = vals.ravel()[order].astype(np.float32)
    gated *= g[:, None]
    ord2 = np.argsort(tok_of, kind="stable")      # token-major, expert asc
    routed = gated[ord2].reshape(T, TOPK, D).sum(axis=1, dtype=np.float32)

    out = shared_out + routed + xf
    return out.reshape(B, Sq, D).astype(np.float32)


kernel.last_result = None



# revision 3
# speedup vs baseline: 1.5071x; 1.5071x over previous
"""MoE (63 routed experts top-7 + 1 shared expert) Trainium2 kernel.

Strategy: expert-parallel sparse dispatch. The router (softmax + top-k,
~0.3% of FLOPs) runs on host; tokens are gathered expert-major into
fixed-capacity weight slots, which are distributed across 8 NeuronCores.
Each core runs an identical (SPMD) Bass program with 9 slots:

- 8 routed slots in fp8-e4m3 with DoubleRow matmuls (2 fp8 weights per
  PE cell -> 2x MACs/cycle). Routed outputs are damped by gates
  (sum of top-7 gates <= ~0.54), so fp8 error stays ~4e-3 relative.
  Weights are pre-scaled x64 on host (w std 0.02 would sit in e4m3's
  subnormal range); the activation's pre-scale 1/64 dequantizes.
- 1 fp16 shared-expert slot (ungated output needs the accuracy);
  8 cores x 1024 tokens covers all 8192 tokens exactly.

Feature-major layout (features on partitions, tokens on the free dim)
so weights need no transpose and biases ride the activation unit's
per-partition bias port. Outputs are gathered and gate-weighted back
on host in the reference's exact accumulation order.
"""

import os
import sys
import math

sys.path.insert(0, "/opt/trn_rl_repo")

import numpy as np

D = 1280          # model dim
I = 1280          # expert inter dim
EXPERTS = 63      # routed experts
TOPK = 7          # routed top-k
CAP = 1024        # tokens per weight slot
CHUNK = 512       # tokens per matmul
CPS = CAP // CHUNK
KT = D // 128     # 10 contraction tiles of 128
NCORES = 8
WSCALE = 64.0     # fp8 weight pre-scale (dequantized in activation)

_PROGRAM_CACHE = {}


# ----------------------------------------------------------------- router

def _route(x2d, wr, br):
    """f32 softmax + top-k, matching jax.nn.softmax / jax.lax.top_k."""
    logits = (x2d @ wr + br).astype(np.float32)
    logits -= logits.max(-1, keepdims=True)
    np.exp(logits, out=logits)
    aff = logits / logits.sum(-1, keepdims=True)
    idx = np.argsort(-aff, axis=-1, kind="stable")[:, :TOPK]
    vals = np.take_along_axis(aff, idx, axis=-1)
    return idx.astype(np.int32), vals.astype(np.float32)


def _build_plan(T, idx):
    """Pack (token, expert) pairs expert-major into CAP-token routed
    pieces. Returns pieces, expert-major order, token-of-pair map."""
    flat = idx.ravel()
    order = np.argsort(flat, kind="stable")          # expert-major slot order
    tok_of = (order // TOPK).astype(np.int64)
    counts = np.bincount(flat, minlength=EXPERTS)
    offs = np.concatenate([[0], np.cumsum(counts)])

    pieces = []  # (expert, a, b)  [a:b) into the expert-major order
    for e in range(EXPERTS):
        a, b = int(offs[e]), int(offs[e + 1])
        while a < b:
            n = min(CAP, b - a)
            pieces.append((e, a, a + n))
            a += n
    return pieces, order, tok_of


# ----------------------------------------------------------- device program

def _build_program(S8):
    import concourse.bass as bass
    import concourse.mybir as mybir
    import concourse.tile as tile
    from concourse import bacc

    f32 = mybir.dt.float32
    f16 = mybir.dt.float16
    f8 = mybir.dt.float8e4
    M8 = S8 * CAP
    M = M8 + CAP  # + one shared slot

    nc = bacc.Bacc("TRN2", target_bir_lowering=False, debug=False,
                   enable_asserts=False, num_devices=NCORES)
    x8T = nc.dram_tensor("x8T", [KT, 128, M8], f8, kind="ExternalInput").ap()
    x16T = nc.dram_tensor("x16T", [KT, 128, CAP], f16, kind="ExternalInput").ap()
    w1s = nc.dram_tensor("w1s", [S8, KT, 128, KT, 128], f8, kind="ExternalInput").ap()
    w2s = nc.dram_tensor("w2s", [S8, KT, 128, KT, 128], f8, kind="ExternalInput").ap()
    b1s = nc.dram_tensor("b1s", [S8, 128, KT], f32, kind="ExternalInput").ap()
    b2s = nc.dram_tensor("b2s", [S8, 128, KT], f32, kind="ExternalInput").ap()
    sw1 = nc.dram_tensor("sw1", [KT, 128, KT, 128], f16, kind="ExternalInput").ap()
    sw2 = nc.dram_tensor("sw2", [KT, 128, KT, 128], f16, kind="ExternalInput").ap()
    sb1 = nc.dram_tensor("sb1", [128, KT], f32, kind="ExternalInput").ap()
    sb2 = nc.dram_tensor("sb2", [128, KT], f32, kind="ExternalInput").ap()
    yT = nc.dram_tensor("yT", [KT, 128, M], f16, kind="ExternalOutput").ap()

    Gelu = mybir.ActivationFunctionType.Gelu
    Ident = mybir.ActivationFunctionType.Identity
    DR = mybir.MatmulPerfMode.DoubleRow
    DQ = 1.0 / WSCALE

    with tile.TileContext(nc) as tc:
        with (
            tc.tile_pool(name="xa", bufs=3) as xa,
            tc.tile_pool(name="w1p", bufs=4) as w1p,
            tc.tile_pool(name="w2p", bufs=4) as w2p,
            tc.tile_pool(name="hp", bufs=3) as hp,
            tc.tile_pool(name="yo", bufs=6) as yo,
            tc.tile_pool(name="bp", bufs=2) as bp,
            tc.tile_pool(name="xa6", bufs=2) as xa6,
            tc.tile_pool(name="w1p6", bufs=4) as w1p6,
            tc.tile_pool(name="w2p6", bufs=4) as w2p6,
            tc.tile_pool(name="hp6", bufs=2) as hp6,
            tc.tile_pool(name="ps", bufs=8, space="PSUM") as ps,
        ):
            # ---- 8 routed fp8 DoubleRow slots -------------------------
            for s in range(S8):
                col0 = s * CAP
                b1t = bp.tile([128, KT], f32, tag="b1", name="b1t")
                nc.sync.dma_start(out=b1t[:, :], in_=b1s[s])
                b2t = bp.tile([128, KT], f32, tag="b2", name="b2t")
                nc.sync.dma_start(out=b2t[:, :], in_=b2s[s])

                xc = []
                for c in range(CPS):
                    xt = xa.tile([128, KT, CHUNK], f8, tag="x", name="xt")
                    for k in range(KT):
                        # SWDGE: keeps HWDGE free for the slot's weight loads
                        nc.gpsimd.dma_start(
                            out=xt[:, k, :],
                            in_=x8T[k, :, col0 + c * CHUNK: col0 + (c + 1) * CHUNK])
                    xc.append(xt)

                hc = [hp.tile([128, KT, CHUNK], f8, tag="h", name=f"h{c}")
                      for c in range(CPS)]

                # layer 1: h = gelu((x @ w1s)/64 + b1)
                for io in range(KT):
                    w1t = w1p.tile([128, KT, 128], f8, tag="w1", name="w1t")
                    nc.sync.dma_start(out=w1t[:, :, :], in_=w1s[s, io])
                    pts = [ps.tile([128, CHUNK], f32, tag=f"ps{c}", bufs=4, name="pt")
                           for c in range(CPS)]
                    for k in range(0, KT, 2):
                        for c in range(CPS):
                            nc.tensor.matmul(pts[c][:, :], w1t[:, k:k + 2, :],
                                             xc[c][:, k:k + 2, :],
                                             start=(k == 0), stop=(k == KT - 2),
                                             perf_mode=DR)
                    for c in range(CPS):
                        nc.scalar.activation(hc[c][:, io, :], pts[c][:, :], Gelu,
                                             bias=b1t[:, io:io + 1], scale=DQ)

                # layer 2: y = (h @ w2s)/64 + b2
                for io in range(KT):
                    w2t = w2p.tile([128, KT, 128], f8, tag="w2", name="w2t")
                    nc.sync.dma_start(out=w2t[:, :, :], in_=w2s[s, io])
                    pts = [ps.tile([128, CHUNK], f32, tag=f"ps{c}", bufs=4, name="pt")
                           for c in range(CPS)]
                    for k in range(0, KT, 2):
                        for c in range(CPS):
                            nc.tensor.matmul(pts[c][:, :], w2t[:, k:k + 2, :],
                                             hc[c][:, k:k + 2, :],
                                             start=(k == 0), stop=(k == KT - 2),
                                             perf_mode=DR)
                    for c in range(CPS):
                        yt = yo.tile([128, CHUNK], f16, tag="y", name="yt")
                        nc.scalar.activation(yt[:, :], pts[c][:, :], Ident,
                                             bias=b2t[:, io:io + 1], scale=DQ)
                        nc.sync.dma_start(
                            out=yT[io, :, col0 + c * CHUNK: col0 + (c + 1) * CHUNK],
                            in_=yt[:, :])

            # ---- 1 shared fp16 slot ----------------------------------
            col0 = S8 * CAP
            sb1t = bp.tile([128, KT], f32, tag="b1", name="sb1t")
            nc.sync.dma_start(out=sb1t[:, :], in_=sb1)
            sb2t = bp.tile([128, KT], f32, tag="b2", name="sb2t")
            nc.sync.dma_start(out=sb2t[:, :], in_=sb2)

            xc = []
            for c in range(CPS):
                xt = xa6.tile([128, KT, CHUNK], f16, tag="x", name="xt6")
                for k in range(KT):
                    nc.gpsimd.dma_start(
                        out=xt[:, k, :],
                        in_=x16T[k, :, c * CHUNK: (c + 1) * CHUNK])
                xc.append(xt)

            hc = [hp6.tile([128, KT, CHUNK], f16, tag="h", name=f"h6{c}")
                  for c in range(CPS)]

            for io in range(KT):
                w1t = w1p6.tile([128, KT, 128], f16, tag="w1", name="w1t6")
                nc.sync.dma_start(out=w1t[:, :, :], in_=sw1[io])
                for c in range(CPS):
                    pt = ps.tile([128, CHUNK], f32, tag="ps0", bufs=4, name="pt")
                    for k in range(KT):
                        nc.tensor.matmul(pt[:, :], w1t[:, k, :], xc[c][:, k, :],
                                         start=(k == 0), stop=(k == KT - 1))
                    nc.scalar.activation(hc[c][:, io, :], pt[:, :], Gelu,
                                         bias=sb1t[:, io:io + 1])

            for io in range(KT):
                w2t = w2p6.tile([128, KT, 128], f16, tag="w2", name="w2t6")
                nc.sync.dma_start(out=w2t[:, :, :], in_=sw2[io])
                for c in range(CPS):
                    pt = ps.tile([128, CHUNK], f32, tag="ps0", bufs=4, name="pt")
                    for k in range(KT):
                        nc.tensor.matmul(pt[:, :], w2t[:, k, :], hc[c][:, k, :],
                                         start=(k == 0), stop=(k == KT - 1))
                    yt = yo.tile([128, CHUNK], f16, tag="y", name="yt")
                    nc.scalar.activation(yt[:, :], pt[:, :], Ident,
                                         bias=sb2t[:, io:io + 1])
                    nc.sync.dma_start(
                        out=yT[io, :, col0 + c * CHUNK: col0 + (c + 1) * CHUNK],
                        in_=yt[:, :])
    nc.compile()
    return nc


def _get_program(S8):
    if S8 not in _PROGRAM_CACHE:
        _PROGRAM_CACHE[S8] = _build_program(S8)
    return _PROGRAM_CACHE[S8]


# ------------------------------------------------------------------ kernel

def _f8():
    import ml_dtypes
    return ml_dtypes.float8_e4m3   # TRN FP8_EXP4: max 240, IEEE-style


def _arrange_w(w):
    """[D, I] -> [io, p, ko, c] so each (slot, io) block DMAs contiguously
    into an SBUF tile laid out [partition, ko, col]."""
    return np.ascontiguousarray(
        w.reshape(KT, 128, KT, 128).transpose(2, 1, 0, 3))


def kernel(x, sw1, sb1, sw2, sb2, rw1, rb1, rw2, rb2, wr, br, _trace=False):
    from concourse.bass_utils import run_bass_kernel_spmd

    f8dt = _f8()
    x = np.asarray(x, dtype=np.float32)
    B, Sq, _ = x.shape
    T = B * Sq
    xf = np.ascontiguousarray(x.reshape(T, D))

    idx, vals = _route(xf, np.asarray(wr, np.float32), np.asarray(br, np.float32))
    pieces, order, tok_of = _build_plan(T, idx)
    R = len(pieces)
    S8 = math.ceil(R / NCORES)
    M8 = S8 * CAP
    assert T == NCORES * CAP, "shared slot layout assumes 8192 tokens"

    rw1 = np.asarray(rw1, np.float32); rw2 = np.asarray(rw2, np.float32)
    rb1 = np.asarray(rb1, np.float32); rb2 = np.asarray(rb2, np.float32)
    sw1 = np.asarray(sw1, np.float32); sw2 = np.asarray(sw2, np.float32)
    sb1 = np.asarray(sb1, np.float32); sb2 = np.asarray(sb2, np.float32)

    w1a = [_arrange_w(rw1[e] * WSCALE).astype(f8dt) for e in range(EXPERTS)]
    w2a = [_arrange_w(rw2[e] * WSCALE).astype(f8dt) for e in range(EXPERTS)]
    b1a = [np.ascontiguousarray(rb1[e].reshape(KT, 128).T) for e in range(EXPERTS)]
    b2a = [np.ascontiguousarray(rb2[e].reshape(KT, 128).T) for e in range(EXPERTS)]
    sw1a = _arrange_w(sw1).astype(np.float16)
    sw2a = _arrange_w(sw2).astype(np.float16)
    sb1a = np.ascontiguousarray(sb1.reshape(KT, 128).T)
    sb2a = np.ascontiguousarray(sb2.reshape(KT, 128).T)

    xfT = np.ascontiguousarray(xf.T)          # [D, T] f32
    xfT8 = xfT.astype(f8dt)
    xfT16 = xfT.astype(np.float16)

    in_maps = []
    for core in range(NCORES):
        x8_core = np.zeros((D, M8), dtype=f8dt)
        w1_core = np.zeros((S8, KT, 128, KT, 128), dtype=f8dt)
        w2_core = np.zeros((S8, KT, 128, KT, 128), dtype=f8dt)
        b1_core = np.zeros((S8, 128, KT), dtype=np.float32)
        b2_core = np.zeros((S8, 128, KT), dtype=np.float32)
        for j in range(S8):
            p = core * S8 + j
            if p >= R:
                continue  # dummy slot: zero weights -> zero output
            e, a, b = pieces[p]
            x8_core[:, j * CAP: j * CAP + (b - a)] = xfT8[:, tok_of[a:b]]
            w1_core[j] = w1a[e]; w2_core[j] = w2a[e]
            b1_core[j] = b1a[e]; b2_core[j] = b2a[e]
        x16_core = xfT16[:, core * CAP: (core + 1) * CAP]
        in_maps.append({
            "x8T": x8_core.reshape(KT, 128, M8),
            "x16T": np.ascontiguousarray(x16_core).reshape(KT, 128, CAP),
            "w1s": w1_core, "w2s": w2_core,
            "b1s": b1_core, "b2s": b2_core,
            "sw1": sw1a, "sw2": sw2a, "sb1": sb1a, "sb2": sb2a,
        })

    nc = _get_program(S8)
    res = run_bass_kernel_spmd(nc, in_maps, core_ids=list(range(NCORES)),
                               trace=_trace)
    kernel.last_result = res

    TK = T * TOPK
    M = M8 + CAP
    gated = np.empty((TK, D), dtype=np.float32)   # expert-major rows
    shared_out = np.empty((T, D), dtype=np.float32)
    for core in range(NCORES):
        Y = res.results[core]["yT"].astype(np.float32).reshape(D, M)
        for j in range(S8):
            p = core * S8 + j
            if p >= R:
                continue
            e, a, b = pieces[p]
            gated[a:b] = Y[:, j * CAP: j * CAP + (b - a)].T
        shared_out[core * CAP: (core + 1) * CAP] = Y[:, M8:].T

    g = vals.ravel()[order].astype(np.float32)
    gated *= g[:, None]
    ord2 = np.argsort(tok_of, kind="stable")      # token-major, expert asc
    routed = gated[ord2].reshape(T, TOPK, D).sum(axis=1, dtype=np.float32)

    out = shared_out + routed + xf
    return out.reshape(B, Sq, D).astype(np.float32)


kernel.last_result = None


# revision 8
# speedup vs baseline: 1.7588x; 1.1670x over previous
"""MoE (63 routed experts top-7 + 1 shared expert) Trainium2 kernel.

Strategy: expert-parallel sparse dispatch. The router (softmax + top-k,
~0.3% of FLOPs) runs on host; tokens are gathered expert-major into
fixed-capacity weight slots, which are distributed across 8 NeuronCores.
Each core runs an identical (SPMD) Bass program with 9 slots:

- 8 routed slots in fp8-e4m3 with DoubleRow matmuls (2 fp8 weights per
  PE cell -> 2x MACs/cycle). Routed outputs are damped by gates
  (sum of top-7 gates <= ~0.54), so fp8 error stays ~4e-3 relative.
  Weights are pre-scaled x64 on host (w std 0.02 would sit in e4m3's
  subnormal range); the activation's pre-scale 1/64 dequantizes.
- 1 fp16 shared-expert slot (ungated output needs the accuracy);
  8 cores x 1024 tokens covers all 8192 tokens exactly.

Feature-major layout (features on partitions, tokens on the free dim)
so weights need no transpose and biases ride the activation unit's
per-partition bias port. Outputs are gathered and gate-weighted back
on host in the reference's exact accumulation order.
"""

import os
import sys
import math

sys.path.insert(0, "/opt/trn_rl_repo")

import numpy as np

D = 1280          # model dim
I = 1280          # expert inter dim
EXPERTS = 63      # routed experts
TOPK = 7          # routed top-k
CAP = 1024        # tokens per weight slot
CHUNK = 512       # tokens per matmul
CPS = CAP // CHUNK
KT = D // 128     # 10 contraction tiles of 128
NCORES = 8
WSCALE = 64.0     # fp8 weight pre-scale (dequantized in activation)

_PROGRAM_CACHE = {}


# ----------------------------------------------------------------- router

def _route(x2d, wr, br):
    """f32 softmax + top-k, matching jax.nn.softmax / jax.lax.top_k."""
    logits = (x2d @ wr + br).astype(np.float32)
    logits -= logits.max(-1, keepdims=True)
    np.exp(logits, out=logits)
    aff = logits / logits.sum(-1, keepdims=True)
    idx = np.argsort(-aff, axis=-1, kind="stable")[:, :TOPK]
    vals = np.take_along_axis(aff, idx, axis=-1)
    return idx.astype(np.int32), vals.astype(np.float32)


def _build_plan(T, idx):
    """Pack (token, expert) pairs expert-major into CAP-token routed
    pieces. Returns pieces, expert-major order, token-of-pair map."""
    flat = idx.ravel()
    order = np.argsort(flat, kind="stable")          # expert-major slot order
    tok_of = (order // TOPK).astype(np.int64)
    counts = np.bincount(flat, minlength=EXPERTS)
    offs = np.concatenate([[0], np.cumsum(counts)])

    pieces = []  # (expert, a, b)  [a:b) into the expert-major order
    for e in range(EXPERTS):
        a, b = int(offs[e]), int(offs[e + 1])
        while a < b:
            n = min(CAP, b - a)
            pieces.append((e, a, a + n))
            a += n
    return pieces, order, tok_of


# ----------------------------------------------------------- device program

def _build_program(S8, zb):
    import concourse.bass as bass
    import concourse.mybir as mybir
    import concourse.tile as tile
    from concourse import bacc

    f32 = mybir.dt.float32
    f16 = mybir.dt.float16
    f8 = mybir.dt.float8e4
    M8 = S8 * CAP
    M = M8 + CAP  # + one shared slot

    nc = bacc.Bacc("TRN2", target_bir_lowering=False, debug=False,
                   enable_asserts=False, num_devices=NCORES)
    mult, add = mybir.AluOpType.mult, mybir.AluOpType.add
    x8T = nc.dram_tensor("x8T", [KT, 128, M8], f8, kind="ExternalInput").ap()
    x16T = nc.dram_tensor("x16T", [KT, 128, CAP], f16, kind="ExternalInput").ap()
    w1s = nc.dram_tensor("w1s", [S8, KT, 128, KT, 128], f8, kind="ExternalInput").ap()
    w2s = nc.dram_tensor("w2s", [S8, KT, 128, KT, 128], f8, kind="ExternalInput").ap()
    if not zb:
        b1s = nc.dram_tensor("b1s", [S8, 128, KT], f32, kind="ExternalInput").ap()
        b2s = nc.dram_tensor("b2s", [S8, 128, KT], f32, kind="ExternalInput").ap()
    sw1 = nc.dram_tensor("sw1", [KT, 128, KT, 128], f16, kind="ExternalInput").ap()
    sw2 = nc.dram_tensor("sw2", [KT, 128, KT, 128], f16, kind="ExternalInput").ap()
    if not zb:
        sb1 = nc.dram_tensor("sb1", [128, KT], f32, kind="ExternalInput").ap()
        sb2 = nc.dram_tensor("sb2", [128, KT], f32, kind="ExternalInput").ap()
    yT = nc.dram_tensor("yT", [KT, 128, M], f16, kind="ExternalOutput").ap()

    Gelu = mybir.ActivationFunctionType.Gelu
    Ident = mybir.ActivationFunctionType.Identity
    DR = mybir.MatmulPerfMode.DoubleRow
    DQ = 1.0 / WSCALE

    with tile.TileContext(nc) as tc:
        with (
            tc.tile_pool(name="xa", bufs=6) as xa,
            tc.tile_pool(name="w1p", bufs=6) as w1p,
            tc.tile_pool(name="w2p", bufs=6) as w2p,
            tc.tile_pool(name="hp", bufs=4) as hp,
            tc.tile_pool(name="yo", bufs=8) as yo,
            tc.tile_pool(name="bp", bufs=2) as bp,
            tc.tile_pool(name="xa6", bufs=2) as xa6,
            tc.tile_pool(name="w1p6", bufs=4) as w1p6,
            tc.tile_pool(name="w2p6", bufs=4) as w2p6,
            tc.tile_pool(name="hp6", bufs=2) as hp6,
            tc.tile_pool(name="ps", bufs=8, space="PSUM") as ps,
        ):
            # ---- 8 routed fp8 DoubleRow slots -------------------------
            for s in range(S8):
                col0 = s * CAP
                if not zb:
                    b1t = bp.tile([128, KT], f32, tag="b1", name="b1t")
                    nc.sync.dma_start(out=b1t[:, :], in_=b1s[s])
                    b2t = bp.tile([128, KT], f32, tag="b2", name="b2t")
                    nc.sync.dma_start(out=b2t[:, :], in_=b2s[s])

                xc = []
                for c in range(CPS):
                    xt = xa.tile([128, KT, CHUNK], f8, tag="x", name="xt")
                    for k in range(KT):
                        # SWDGE: keeps HWDGE free for the slot's weight loads
                        nc.gpsimd.dma_start(
                            out=xt[:, k, :],
                            in_=x8T[k, :, col0 + c * CHUNK: col0 + (c + 1) * CHUNK])
                    xc.append(xt)

                hc = [hp.tile([128, KT, CHUNK], f8, tag="h", name=f"h{c}")
                      for c in range(CPS)]

                # layer 1: h = gelu((x @ w1s)/64 + b1)
                for io in range(KT):
                    w1t = w1p.tile([128, KT, 128], f8, tag="w1", name="w1t")
                    nc.sync.dma_start(out=w1t[:, :, :], in_=w1s[s, io])
                    pts = [ps.tile([128, CHUNK], f32, tag=f"ps{c}", bufs=4, name="pt")
                           for c in range(CPS)]
                    for k in range(0, KT, 2):
                        for c in range(CPS):
                            nc.tensor.matmul(pts[c][:, :], w1t[:, k:k + 2, :],
                                             xc[c][:, k:k + 2, :],
                                             start=(k == 0), stop=(k == KT - 2),
                                             perf_mode=DR)
                    for c in range(CPS):
                        if zb:
                            nc.scalar.activation(hc[c][:, io, :], pts[c][:, :],
                                                 Gelu, scale=DQ)
                        else:
                            nc.scalar.activation(hc[c][:, io, :], pts[c][:, :],
                                                 Gelu, bias=b1t[:, io:io + 1],
                                                 scale=DQ)

                # layer 2: y = (h @ w2s)/64 + b2
                for io in range(KT):
                    w2t = w2p.tile([128, KT, 128], f8, tag="w2", name="w2t")
                    nc.sync.dma_start(out=w2t[:, :, :], in_=w2s[s, io])
                    pts = [ps.tile([128, CHUNK], f32, tag=f"ps{c}", bufs=4, name="pt")
                           for c in range(CPS)]
                    for k in range(0, KT, 2):
                        for c in range(CPS):
                            nc.tensor.matmul(pts[c][:, :], w2t[:, k:k + 2, :],
                                             hc[c][:, k:k + 2, :],
                                             start=(k == 0), stop=(k == KT - 2),
                                             perf_mode=DR)
                    for c in range(CPS):
                        yt = yo.tile([128, CHUNK], f16, tag="y", name="yt")
                        # DVE: y = psum*(1/64) [+ b2]  (keeps ScalarE free
                        # for the Gelus, which only it can run)
                        if zb:
                            nc.vector.tensor_scalar_mul(yt[:, :], pts[c][:, :], DQ)
                        else:
                            nc.vector.tensor_scalar(
                                yt[:, :], pts[c][:, :], DQ, b2t[:, io:io + 1],
                                op0=mult, op1=add)
                        nc.sync.dma_start(
                            out=yT[io, :, col0 + c * CHUNK: col0 + (c + 1) * CHUNK],
                            in_=yt[:, :])

            # ---- 1 shared fp16 slot ----------------------------------
            col0 = S8 * CAP
            if not zb:
                sb1t = bp.tile([128, KT], f32, tag="b1", name="sb1t")
                nc.sync.dma_start(out=sb1t[:, :], in_=sb1)
                sb2t = bp.tile([128, KT], f32, tag="b2", name="sb2t")
                nc.sync.dma_start(out=sb2t[:, :], in_=sb2)

            xc = []
            for c in range(CPS):
                xt = xa6.tile([128, KT, CHUNK], f16, tag="x", name="xt6")
                for k in range(KT):
                    nc.gpsimd.dma_start(
                        out=xt[:, k, :],
                        in_=x16T[k, :, c * CHUNK: (c + 1) * CHUNK])
                xc.append(xt)

            hc = [hp6.tile([128, KT, CHUNK], f16, tag="h", name=f"h6{c}")
                  for c in range(CPS)]

            for io in range(KT):
                w1t = w1p6.tile([128, KT, 128], f16, tag="w1", name="w1t6")
                nc.sync.dma_start(out=w1t[:, :, :], in_=sw1[io])
                for c in range(CPS):
                    pt = ps.tile([128, CHUNK], f32, tag="ps0", bufs=4, name="pt")
                    for k in range(KT):
                        nc.tensor.matmul(pt[:, :], w1t[:, k, :], xc[c][:, k, :],
                                         start=(k == 0), stop=(k == KT - 1))
                    if zb:
                        nc.scalar.activation(hc[c][:, io, :], pt[:, :], Gelu)
                    else:
                        nc.scalar.activation(hc[c][:, io, :], pt[:, :], Gelu,
                                             bias=sb1t[:, io:io + 1])

            for io in range(KT):
                w2t = w2p6.tile([128, KT, 128], f16, tag="w2", name="w2t6")
                nc.sync.dma_start(out=w2t[:, :, :], in_=sw2[io])
                for c in range(CPS):
                    pt = ps.tile([128, CHUNK], f32, tag="ps0", bufs=4, name="pt")
                    for k in range(KT):
                        nc.tensor.matmul(pt[:, :], w2t[:, k, :], hc[c][:, k, :],
                                         start=(k == 0), stop=(k == KT - 1))
                    yt = yo.tile([128, CHUNK], f16, tag="y", name="yt")
                    if zb:
                        nc.vector.tensor_scalar_mul(yt[:, :], pt[:, :], 1.0)
                    else:
                        nc.vector.tensor_scalar(
                            yt[:, :], pt[:, :], 1.0, sb2t[:, io:io + 1],
                            op0=mult, op1=add)
                    nc.sync.dma_start(
                        out=yT[io, :, col0 + c * CHUNK: col0 + (c + 1) * CHUNK],
                        in_=yt[:, :])
    nc.compile()
    return nc


def _get_program(S8, zb):
    key = (S8, zb)
    if key not in _PROGRAM_CACHE:
        _PROGRAM_CACHE[key] = _build_program(S8, zb)
    return _PROGRAM_CACHE[key]


# ------------------------------------------------------------------ kernel

def _f8():
    import ml_dtypes
    return ml_dtypes.float8_e4m3   # TRN FP8_EXP4: max 240, IEEE-style


def _arrange_w(w):
    """[D, I] -> [io, p, ko, c] so each (slot, io) block DMAs contiguously
    into an SBUF tile laid out [partition, ko, col]."""
    return np.ascontiguousarray(
        w.reshape(KT, 128, KT, 128).transpose(2, 1, 0, 3))


def kernel(x, sw1, sb1, sw2, sb2, rw1, rb1, rw2, rb2, wr, br, _trace=False):
    from concourse.bass_utils import run_bass_kernel_spmd

    f8dt = _f8()
    x = np.asarray(x, dtype=np.float32)
    B, Sq, _ = x.shape
    T = B * Sq
    xf = np.ascontiguousarray(x.reshape(T, D))

    idx, vals = _route(xf, np.asarray(wr, np.float32), np.asarray(br, np.float32))
    pieces, order, tok_of = _build_plan(T, idx)
    R = len(pieces)
    S8 = math.ceil(R / NCORES)
    M8 = S8 * CAP
    assert T == NCORES * CAP, "shared slot layout assumes 8192 tokens"

    rw1 = np.asarray(rw1, np.float32); rw2 = np.asarray(rw2, np.float32)
    rb1 = np.asarray(rb1, np.float32); rb2 = np.asarray(rb2, np.float32)
    sw1 = np.asarray(sw1, np.float32); sw2 = np.asarray(sw2, np.float32)
    sb1 = np.asarray(sb1, np.float32); sb2 = np.asarray(sb2, np.float32)
    zb = not (rb1.any() or rb2.any() or sb1.any() or sb2.any())

    w1a = [_arrange_w(rw1[e] * WSCALE).astype(f8dt) for e in range(EXPERTS)]
    w2a = [_arrange_w(rw2[e] * WSCALE).astype(f8dt) for e in range(EXPERTS)]
    b1a = [np.ascontiguousarray(rb1[e].reshape(KT, 128).T) for e in range(EXPERTS)]
    b2a = [np.ascontiguousarray(rb2[e].reshape(KT, 128).T) for e in range(EXPERTS)]
    sw1a = _arrange_w(sw1).astype(np.float16)
    sw2a = _arrange_w(sw2).astype(np.float16)
    sb1a = np.ascontiguousarray(sb1.reshape(KT, 128).T)
    sb2a = np.ascontiguousarray(sb2.reshape(KT, 128).T)

    xfT = np.ascontiguousarray(xf.T)          # [D, T] f32
    xfT8 = xfT.astype(f8dt)
    xfT16 = xfT.astype(np.float16)

    in_maps = []
    for core in range(NCORES):
        x8_core = np.zeros((D, M8), dtype=f8dt)
        w1_core = np.zeros((S8, KT, 128, KT, 128), dtype=f8dt)
        w2_core = np.zeros((S8, KT, 128, KT, 128), dtype=f8dt)
        b1_core = np.zeros((S8, 128, KT), dtype=np.float32)
        b2_core = np.zeros((S8, 128, KT), dtype=np.float32)
        for j in range(S8):
            p = core * S8 + j
            if p >= R:
                continue  # dummy slot: zero weights -> zero output
            e, a, b = pieces[p]
            x8_core[:, j * CAP: j * CAP + (b - a)] = xfT8[:, tok_of[a:b]]
            w1_core[j] = w1a[e]; w2_core[j] = w2a[e]
            b1_core[j] = b1a[e]; b2_core[j] = b2a[e]
        x16_core = xfT16[:, core * CAP: (core + 1) * CAP]
        im = {
            "x8T": x8_core.reshape(KT, 128, M8),
            "x16T": np.ascontiguousarray(x16_core).reshape(KT, 128, CAP),
            "w1s": w1_core, "w2s": w2_core,
            "sw1": sw1a, "sw2": sw2a,
        }
        if not zb:
            im.update({"b1s": b1_core, "b2s": b2_core,
                       "sb1": sb1a, "sb2": sb2a})
        in_maps.append(im)

    nc = _get_program(S8, zb)
    res = run_bass_kernel_spmd(nc, in_maps, core_ids=list(range(NCORES)),
                               trace=_trace)
    kernel.last_result = res

    TK = T * TOPK
    M = M8 + CAP
    gated = np.empty((TK, D), dtype=np.float32)   # expert-major rows
    shared_out = np.empty((T, D), dtype=np.float32)
    for core in range(NCORES):
        Y = res.results[core]["yT"].astype(np.float32).reshape(D, M)
        for j in range(S8):
            p = core * S8 + j
            if p >= R:
                continue
            e, a, b = pieces[p]
            gated[a:b] = Y[:, j * CAP: j * CAP + (b - a)].T
        shared_out[core * CAP: (core + 1) * CAP] = Y[:, M8:].T

    g = vals.ravel()[order].astype(np.float32)
    gated *= g[:, None]
    ord2 = np.argsort(tok_of, kind="stable")      # token-major, expert asc
    routed = gated[ord2].reshape(T, TOPK, D).sum(axis=1, dtype=np.float32)

    out = shared_out + routed + xf
    return out.reshape(B, Sq, D).astype(np.float32)


kernel.last_result = None
